# revision 8
# baseline (speedup 1.0000x reference)
"""AdaptiveGridKANLayer on 8 TRN2 NeuronCores.

out[b,o] = sum_i sum_g exp(-((x[b,i]-c_g)/w)^2) * coeffs[o,i,g]
         + sum_i silu(x[b,i]) * base_w[o,i]

B=65536, in=out=128, G=8, centers = linspace(-1,1,8), w = 2/7.

Strategy (data-parallel over batch, weights replicated):
- Host: transpose x to feature-major [128, B], shard columns 8 ways; fold the
  Gaussian factorization constants e^(7g-g^2) into the coeffs; output comes
  back transposed (bf16) and is upcast/untransposed on host.
- Device, per core (u = (x+1)/w): basis_g = e^(-(u-g)^2) = p * s^g * const
  with p = exp(-u^2) (ScalarE Square+Exp), s = exp(7x) (ScalarE Exp).
  VectorE builds the power chain t_g = t_{g-1} * s (bf16 2x-mode
  multiplies); TensorE contracts 9 K-tiles (8 Gaussian + silu) as
  [128,128]x[128,512] bf16 matmuls accumulating into f32 PSUM.
- Weights DMA first on the sync queue (300KB, ~1us) so matmuls are never
  weight-blocked (behind the x stream they land at ~11us); the PE HAM clock
  gate is warmed by memset-fed matmuls with no DMA deps starting at t~0.
- GpSimd is kept off the elementwise path entirely: its tensor ops share
  the DVE SBUF port and halve VectorE throughput while active.
- silu lives in a different activation-table set than exp, so all Exp/Square
  passes run first, then one table switch, then all Silu passes (gated via a
  bias tile so the scheduler cannot interleave the phases); the silu matmuls
  close each PSUM accumulation group, then PSUM is copied out as bf16
  (copies split between ScalarE and VectorE) and DMAed.
- The first chunk's elementwise work runs at half width so the vector chain
  starts as early as possible.
"""

import numpy as np

BATCH = 65536
GRID = 8
NCORES = 8
BLOC = BATCH // NCORES  # 8192 batch columns per core
FDE = 2048  # elementwise chunk
NCH = BLOC // FDE
FDP = 512  # psum sub-chunk
NSUB = BLOC // FDP
MMF = 512  # matmul free dim (one PSUM bank)
W = 2.0 / (GRID - 1)

_NC = None


def _build():
    import concourse.mybir as mybir
    from concourse import bacc
    from concourse.tile import TileContext, add_dep_helper

    AF = mybir.ActivationFunctionType
    bf16 = mybir.dt.bfloat16
    f32 = mybir.dt.float32

    nc = bacc.Bacc("TRN2", num_devices=NCORES)
    xt = nc.dram_tensor("xt", [128, BLOC], f32, kind="ExternalInput").ap()
    wt = nc.dram_tensor("wt", [128, 9 * 128], bf16, kind="ExternalInput").ap()
    out = nc.dram_tensor("out", [128, BLOC], bf16, kind="ExternalOutput").ap()

    with TileContext(nc) as tc:
        with (
            tc.tile_pool(name="const", bufs=1) as cpool,
            tc.tile_pool(name="work", bufs=2) as wpool,
            tc.tile_pool(name="obuf", bufs=6) as opool,
            tc.tile_pool(name="psum", bufs=8, space="PSUM") as ppool,
        ):
            # Dummy activation with no DMA dependencies: forces the
            # exp_and_others ACT table load to run during the preamble
            # instead of queueing behind the x DMAs.
            warm_act = cpool.tile([128, 1], f32, name="warm_act")
            nc.vector.memset(warm_act[:], 0.0)
            cst = cpool.tile([128, 1], f32, name="cst")
            nc.vector.memset(cst[:], 1.0 / W)
            nc.scalar.activation(warm_act[:], warm_act[:], AF.Exp, scale=1.0)

            # PE HAM clock warm: memset-fed matmuls (no DMA deps) keep the
            # PE stream alive from t~0 so the clock gate is fully open
            # before the first real matmul.
            wm_s = cpool.tile([128, 128], bf16, name="wm_s")
            wm_m = cpool.tile([128, MMF], bf16, name="wm_m")
            nc.vector.memset(wm_s[:], 0.25)
            nc.vector.memset(wm_m[:], 0.25)
            warm_ps = ppool.tile([128, FDP], f32, name="warm_ps", tag="psum")
            for _ in range(12):
                nc.tensor.matmul(
                    warm_ps[:], wm_s[:], wm_m[:], start=True, stop=True
                )

            # weights first (300KB, ~1us) so matmuls are never
            # weight-blocked, then the x stream.
            w_sb = cpool.tile([128, 9, 128], bf16, name="w_sb")
            nc.sync.dma_start(w_sb[:], wt.rearrange("p (g o) -> p g o", g=9))
            x_all = cpool.tile([128, BLOC], f32, name="x_all")
            xpieces = [512, 512, 1024] + [1024] * ((BLOC - 2048) // 1024)
            lo = 0
            for w in xpieces:
                nc.sync.dma_start(x_all[:, lo : lo + w], xt[:, lo : lo + w])
                lo += w

            # Phase 1 (exp_and_others): s, q, t0 + vector chain + gauss MMs.
            # Graded chunk sizes: big chunks amortize per-op overhead, the
            # small final chunks shorten the serial matmul tail behind the
            # last chain op.
            CH = [(0, 2048), (2048, 2048), (4096, 2048), (6144, 1024), (7168, 1024)]
            psums = []
            last_t0 = None
            for ci, (cbase, clen) in enumerate(CH):
                s = wpool.tile([128, 1, FDE], bf16, tag="s", bufs=3, name=f"s_{ci}")
                q = wpool.tile([128, FDE], f32, tag="q", name=f"q_{ci}")
                # all 8 chain levels live in one tile so a single in-place
                # DVE op (out = levels 1-7, in = levels 0-6, stream order)
                # computes the whole power chain.
                T = wpool.tile(
                    [128, GRID, FDE], bf16, tag="tg", bufs=3, name=f"tg_{ci}"
                )
                tg = [T[:, g, :] for g in range(GRID)]
                # chunk 0 runs at narrow widths so the chain starts earlier
                widths = [512, 512, 1024] if ci == 0 else [clen]
                off = 0
                for fd in widths:
                    hs = slice(off, off + fd)
                    xc = x_all[:, cbase + off : cbase + off + fd]
                    off += fd
                    nc.scalar.activation(s[:, 0, hs], xc, AF.Exp, scale=2.0 / W)
                    nc.scalar.activation(
                        q[:, hs], xc, AF.Square, bias=cst[:], scale=1.0 / W
                    )
                    last_t0 = nc.scalar.activation(
                        T[:, 0, hs], q[:, hs], AF.Exp, scale=-1.0
                    )
                # chain: one mega-op per half-chunk for c0 (earlier matmul
                # start), one full-width op for later chunks.
                chain_pieces = [(0, 1024), (1024, 1024)] if ci == 0 else [(0, clen)]
                for lo, fd in chain_pieces:
                    nc.vector.tensor_mul(
                        T[:, 1:GRID, lo : lo + fd],
                        T[:, 0 : GRID - 1, lo : lo + fd],
                        s[:, :, lo : lo + fd].to_broadcast((128, GRID - 1, fd)),
                    )
                for half in range(clen // FDP):
                    psum = ppool.tile([128, FDP], f32, tag="psum", name="psum")
                    psums.append(psum)
                    for g in range(GRID):
                        for n in range(FDP // MMF):
                            lo = half * FDP + n * MMF
                            nc.tensor.matmul(
                                psum[:, n * MMF : (n + 1) * MMF],
                                w_sb[:, g, :],
                                tg[g][:, lo : lo + MMF],
                                start=(g == 0),
                                stop=False,
                            )

            # Phase 2 (silu_and_others), gated on the last exp-phase act via a
            # bias tile so the two table phases cannot interleave on ScalarE.
            gate = cpool.tile([128, 1], f32)
            gate_op = nc.scalar.activation(
                gate[:], x_all[:, BLOC - 1 : BLOC], AF.Identity, scale=0.0
            )
            add_dep_helper(gate_op.ins, last_t0.ins, True, "table phase order")
            silu_all = cpool.tile([128, BLOC], bf16)
            FDS = 1024  # silu act width: finer so psum groups close sooner
            subs_per_silu = FDS // FDP

            def emit_copy(k, engine):
                ob = opool.tile([128, FDP], bf16, tag="ob", name=f"ob_{k}")
                if engine == "s":
                    nc.scalar.copy(ob[:], psums[k][:])
                else:
                    nc.vector.tensor_copy(ob[:], psums[k][:])
                nc.sync.dma_start(out[:, k * FDP : (k + 1) * FDP], ob[:])

            for j in range(BLOC // FDS):
                js = slice(j * FDS, (j + 1) * FDS)
                nc.scalar.activation(
                    silu_all[:, js], x_all[:, js], AF.Silu, bias=gate[:]
                )
                for half in range(subs_per_silu):
                    k = j * subs_per_silu + half
                    for n in range(FDP // MMF):
                        lo = k * FDP + n * MMF
                        nc.tensor.matmul(
                            psums[k][:, n * MMF : (n + 1) * MMF],
                            w_sb[:, 8, :],
                            silu_all[:, lo : lo + MMF],
                            start=False,
                            stop=True,
                        )
                # copies 0-3 free psum slots for the second-half gauss
                # matmuls, so they run early on ScalarE; the rest follow
                # the silu acts (ScalarE) or the chain (VectorE).
                if j == 1:
                    for k in range(0, 4):
                        emit_copy(k, "s")
            for k in range(4, 8):
                emit_copy(k, "s")
            for k in range(8, NSUB):
                emit_copy(k, "v")


    nc.compile()
    return nc


def _prep_weights(coeffs, base_w):
    import ml_dtypes

    g = np.arange(GRID, dtype=np.float64)
    K = np.exp(7.0 * g - g * g)  # t_g = basis_g * e^(g^2-7g) -> fold inverse
    blocks = [
        (coeffs[:, :, gi].astype(np.float64) * K[gi]).T for gi in range(GRID)
    ]  # [in, out] each
    blocks.append(base_w.astype(np.float64).T)
    wtm = np.concatenate(blocks, axis=1)  # [128, 9*128]
    return np.ascontiguousarray(wtm.astype(ml_dtypes.bfloat16))


def kernel(x, coeffs, base_w, centers):
    from concourse.bass_utils import run_bass_kernel_spmd

    global _NC
    if _NC is None:
        _NC = _build()

    wtm = _prep_weights(coeffs, base_w)
    xT = np.ascontiguousarray(np.asarray(x, dtype=np.float32).T)  # [128, B]
    in_maps = [
        {
            "xt": np.ascontiguousarray(xT[:, c * BLOC : (c + 1) * BLOC]),
            "wt": wtm,
        }
        for c in range(NCORES)
    ]
    res = run_bass_kernel_spmd(_NC, in_maps, list(range(NCORES)))
    outT = np.concatenate(
        [res.results[c]["out"].astype(np.float32) for c in range(NCORES)], axis=1
    )
    return np.ascontiguousarray(outT.T)


# revision 9
# speedup vs baseline: 1.0895x; 1.0895x over previous
"""AdaptiveGridKANLayer on 8 TRN2 NeuronCores.

out[b,o] = sum_i sum_g exp(-((x[b,i]-c_g)/w)^2) * coeffs[o,i,g]
         + sum_i silu(x[b,i]) * base_w[o,i]

B=65536, in=out=128, G=8, centers = linspace(-1,1,8), w = 2/7.

Strategy (data-parallel over batch, weights replicated):
- Host: transpose x to feature-major [128, B], shard columns 8 ways; fold the
  Gaussian factorization constants e^(7g-g^2) into the coeffs; output comes
  back transposed (bf16) and is upcast/untransposed on host.
- Device, per core (u = (x+1)/w): basis_g = e^(-(u-g)^2) = p * s^g * const
  with p = exp(-u^2) (ScalarE Square+Exp), s = exp(7x) (ScalarE Exp).
  VectorE builds the power chain t_g = t_{g-1} * s (bf16 2x-mode
  multiplies); TensorE contracts 9 K-tiles (8 Gaussian + silu) as
  [128,128]x[128,512] bf16 matmuls accumulating into f32 PSUM.
- Weights DMA first on the sync queue (300KB, ~1us) so matmuls are never
  weight-blocked (behind the x stream they land at ~11us); the PE HAM clock
  gate is warmed by memset-fed matmuls with no DMA deps starting at t~0.
- GpSimd is kept off the elementwise path entirely: its tensor ops share
  the DVE SBUF port and halve VectorE throughput while active.
- silu lives in a different activation-table set than exp, so all Exp/Square
  passes run first, then one table switch, then all Silu passes (gated via a
  bias tile so the scheduler cannot interleave the phases); the silu matmuls
  close each PSUM accumulation group, then PSUM is copied out as bf16
  (copies split between ScalarE and VectorE) and DMAed.
- The first chunk's elementwise work runs at half width so the vector chain
  starts as early as possible.
"""

import numpy as np

BATCH = 65536
GRID = 8
NCORES = 8
BLOC = BATCH // NCORES  # 8192 batch columns per core
FDE = 2048  # elementwise chunk
NCH = BLOC // FDE
FDP = 512  # psum sub-chunk
NSUB = BLOC // FDP
MMF = 512  # matmul free dim (one PSUM bank)
W = 2.0 / (GRID - 1)

_NC = None


def _build():
    import concourse.mybir as mybir
    from concourse import bacc
    from concourse.tile import TileContext, add_dep_helper

    AF = mybir.ActivationFunctionType
    bf16 = mybir.dt.bfloat16
    f32 = mybir.dt.float32

    nc = bacc.Bacc("TRN2", num_devices=NCORES)
    cst = nc.alloc_sbuf_tensor("const-float32-bias-c", [128, 1], f32)
    nc.gpsimd.memset(cst.ap(), 1.0 / W)
    nc.const_aps.aps[(f32, 1.0 / W)] = cst.ap()
    nc.all_engine_barrier()
    xt = nc.dram_tensor("xt", [128, BLOC], f32, kind="ExternalInput").ap()
    wt = nc.dram_tensor("wt", [128, 9 * 128], bf16, kind="ExternalInput").ap()
    out = nc.dram_tensor("out", [128, BLOC], bf16, kind="ExternalOutput").ap()

    with TileContext(nc) as tc:
        with (
            tc.tile_pool(name="const", bufs=1) as cpool,
            tc.tile_pool(name="work", bufs=2) as wpool,
            tc.tile_pool(name="obuf", bufs=6) as opool,
            tc.tile_pool(name="psum", bufs=8, space="PSUM") as ppool,
        ):
            # Dummy activation with no DMA dependencies: forces the
            # exp_and_others ACT table load to run during the preamble
            # instead of queueing behind the x DMAs.
            warm_act = cpool.tile([128, 1], f32, name="warm_act")
            nc.gpsimd.memset(warm_act[:], 0.0)
            nc.scalar.activation(warm_act[:], warm_act[:], AF.Exp, scale=1.0)

            # PE HAM clock warm: memset-fed matmuls (no DMA deps) keep the
            # PE stream alive from t~0 so the clock gate is fully open
            # before the first real matmul.
            wm_s = cpool.tile([128, 128], bf16, name="wm_s")
            wm_m = cpool.tile([128, MMF], bf16, name="wm_m")
            nc.gpsimd.memset(wm_s[:], 0.25)
            nc.gpsimd.memset(wm_m[:], 0.25)
            warm_ps = ppool.tile([128, FDP], f32, name="warm_ps", tag="psum")
            for _ in range(12):
                nc.tensor.matmul(
                    warm_ps[:], wm_s[:], wm_m[:], start=True, stop=True
                )

            # weights first (300KB, ~1us) so matmuls are never
            # weight-blocked, then the x stream.
            w_sb = cpool.tile([128, 9, 128], bf16, name="w_sb")
            nc.sync.dma_start(w_sb[:], wt.rearrange("p (g o) -> p g o", g=9))
            x_all = cpool.tile([128, BLOC], f32, name="x_all")
            xpieces = [512, 512, 1024] + [1024] * ((BLOC - 2048) // 1024)
            lo = 0
            for w in xpieces:
                nc.sync.dma_start(x_all[:, lo : lo + w], xt[:, lo : lo + w])
                lo += w

            # Phase 1 (exp_and_others): s, q, t0 + vector chain + gauss MMs.
            psums = []
            last_t0 = None
            for c in range(NCH):
                s = wpool.tile([128, 1, FDE], bf16, tag="s", bufs=3, name=f"s_{c}")
                q = wpool.tile([128, FDE], f32, tag="q", name=f"q_{c}")
                # all 8 chain levels live in one tile so a single in-place
                # DVE op per 1024-block (out = levels 1-7, in = levels 0-6,
                # stream order) computes the whole power chain: 2 DVE ops
                # per chunk instead of 14, at full 2x rate.
                T = wpool.tile(
                    [128, GRID, FDE], bf16, tag="tg", bufs=3, name=f"tg_{c}"
                )
                tg = [T[:, g, :] for g in range(GRID)]
                # chunk 0 runs at narrow widths so the chain starts earlier
                widths = [512, 512, 1024] if c == 0 else [1024, 1024]
                off = 0
                for fd in widths:
                    hs = slice(off, off + fd)
                    xc = x_all[:, c * FDE + off : c * FDE + off + fd]
                    off += fd
                    nc.scalar.activation(s[:, 0, hs], xc, AF.Exp, scale=2.0 / W)
                    nc.scalar.activation(
                        q[:, hs], xc, AF.Square, bias=1.0 / W, scale=1.0 / W
                    )
                    last_t0 = nc.scalar.activation(
                        T[:, 0, hs], q[:, hs], AF.Exp, scale=-1.0
                    )
                    if off % 1024 == 0:
                        lo = off - 1024
                        nc.vector.tensor_mul(
                            T[:, 1:GRID, lo:off],
                            T[:, 0 : GRID - 1, lo:off],
                            s[:, :, lo:off].to_broadcast((128, GRID - 1, 1024)),
                        )
                for half in range(FDE // FDP):
                    psum = ppool.tile([128, FDP], f32, tag="psum", name="psum")
                    psums.append(psum)
                    for g in range(GRID):
                        for n in range(FDP // MMF):
                            lo = half * FDP + n * MMF
                            nc.tensor.matmul(
                                psum[:, n * MMF : (n + 1) * MMF],
                                w_sb[:, g, :],
                                tg[g][:, lo : lo + MMF],
                                start=(g == 0),
                                stop=False,
                            )

            # Phase 2 (silu_and_others), gated on the last exp-phase act via a
            # bias tile so the two table phases cannot interleave on ScalarE.
            gate = cpool.tile([128, 1], f32)
            gate_op = nc.scalar.activation(
                gate[:], x_all[:, BLOC - 1 : BLOC], AF.Identity, scale=0.0
            )
            add_dep_helper(gate_op.ins, last_t0.ins, True, "table phase order")
            silu_all = cpool.tile([128, BLOC], bf16)
            FDS = 1024  # silu act width: finer so psum groups close sooner
            subs_per_silu = FDS // FDP

            def emit_copy(k, engine):
                ob = opool.tile([128, FDP], bf16, tag="ob", name=f"ob_{k}")
                if engine == "s":
                    nc.scalar.copy(ob[:], psums[k][:])
                else:
                    nc.vector.tensor_copy(ob[:], psums[k][:])
                nc.sync.dma_start(out[:, k * FDP : (k + 1) * FDP], ob[:])

            for j in range(BLOC // FDS):
                js = slice(j * FDS, (j + 1) * FDS)
                nc.scalar.activation(
                    silu_all[:, js], x_all[:, js], AF.Silu, bias=gate[:]
                )
                for half in range(subs_per_silu):
                    k = j * subs_per_silu + half
                    for n in range(FDP // MMF):
                        lo = k * FDP + n * MMF
                        nc.tensor.matmul(
                            psums[k][:, n * MMF : (n + 1) * MMF],
                            w_sb[:, 8, :],
                            silu_all[:, lo : lo + MMF],
                            start=False,
                            stop=True,
                        )
                # copies 0-3 free psum slots for the second-half gauss
                # matmuls, so they run early on ScalarE; the rest follow
                # the silu acts (ScalarE) or the chain (VectorE).
                if j == 1:
                    for k in range(0, 4):
                        emit_copy(k, "s")
            for k in range(4, 8):
                emit_copy(k, "s")
            for k in range(8, NSUB):
                emit_copy(k, "v")


    nc.compile()
    return nc


def _prep_weights(coeffs, base_w):
    import ml_dtypes

    g = np.arange(GRID, dtype=np.float64)
    K = np.exp(7.0 * g - g * g)  # t_g = basis_g * e^(g^2-7g) -> fold inverse
    blocks = [
        (coeffs[:, :, gi].astype(np.float64) * K[gi]).T for gi in range(GRID)
    ]  # [in, out] each
    blocks.append(base_w.astype(np.float64).T)
    wtm = np.concatenate(blocks, axis=1)  # [128, 9*128]
    return np.ascontiguousarray(wtm.astype(ml_dtypes.bfloat16))


def kernel(x, coeffs, base_w, centers):
    from concourse.bass_utils import run_bass_kernel_spmd

    global _NC
    if _NC is None:
        _NC = _build()

    wtm = _prep_weights(coeffs, base_w)
    xT = np.ascontiguousarray(np.asarray(x, dtype=np.float32).T)  # [128, B]
    in_maps = [
        {
            "xt": np.ascontiguousarray(xT[:, c * BLOC : (c + 1) * BLOC]),
            "wt": wtm,
        }
        for c in range(NCORES)
    ]
    res = run_bass_kernel_spmd(_NC, in_maps, list(range(NCORES)))
    outT = np.concatenate(
        [res.results[c]["out"].astype(np.float32) for c in range(NCORES)], axis=1
    )
    return np.ascontiguousarray(outT.T)


# revision 10
# speedup vs baseline: 1.1872x; 1.0897x over previous
"""AdaptiveGridKANLayer on 8 TRN2 NeuronCores.

out[b,o] = sum_i sum_g exp(-((x[b,i]-c_g)/w)^2) * coeffs[o,i,g]
         + sum_i silu(x[b,i]) * base_w[o,i]

B=65536, in=out=128, G=8, centers = linspace(-1,1,8), w = 2/7.

Strategy (data-parallel over batch, weights replicated):
- Host: transpose x to feature-major [128, B], shard columns 8 ways; fold the
  Gaussian factorization constants e^(7g-g^2) into the coeffs; output comes
  back transposed (bf16) and is upcast/untransposed on host.
- Device, per core (u = (x+1)/w): basis_g = e^(-(u-g)^2) = p * s^g * const
  with p = exp(-u^2) (ScalarE Square+Exp), s = exp(7x) (ScalarE Exp).
  VectorE builds the power chain t_g = t_{g-1} * s (bf16 2x-mode
  multiplies); TensorE contracts 9 K-tiles (8 Gaussian + silu) as
  [128,128]x[128,512] bf16 matmuls accumulating into f32 PSUM.
- Weights DMA first on the sync queue (300KB, ~1us) so matmuls are never
  weight-blocked (behind the x stream they land at ~11us); the PE HAM clock
  gate is warmed by memset-fed matmuls with no DMA deps starting at t~0.
- GpSimd is kept off the elementwise path entirely: its tensor ops share
  the DVE SBUF port and halve VectorE throughput while active.
- silu lives in a different activation-table set than exp, so all Exp/Square
  passes run first, then one table switch, then all Silu passes (gated via a
  bias tile so the scheduler cannot interleave the phases); the silu matmuls
  close each PSUM accumulation group, then PSUM is copied out as bf16
  (copies split between ScalarE and VectorE) and DMAed.
- The first chunk's elementwise work runs at half width so the vector chain
  starts as early as possible.
"""

import numpy as np

BATCH = 65536
GRID = 8
NCORES = 8
BLOC = BATCH // NCORES  # 8192 batch columns per core
FDE = 2048  # elementwise chunk
NCH = BLOC // FDE
FDP = 512  # psum sub-chunk
NSUB = BLOC // FDP
MMF = 512  # matmul free dim (one PSUM bank)
W = 2.0 / (GRID - 1)

_NC = None


def _build():
    import concourse.mybir as mybir
    from concourse import bacc
    from concourse.tile import TileContext, add_dep_helper

    AF = mybir.ActivationFunctionType
    bf16 = mybir.dt.bfloat16
    f32 = mybir.dt.float32

    nc = bacc.Bacc("TRN2", num_devices=NCORES)
    cst = nc.alloc_sbuf_tensor("const-float32-bias-c", [128, 1], f32)
    nc.gpsimd.memset(cst.ap(), 1.0 / W)
    nc.const_aps.aps[(f32, 1.0 / W)] = cst.ap()
    nc.all_engine_barrier()
    xt = nc.dram_tensor("xt", [128, BLOC], f32, kind="ExternalInput").ap()
    wt = nc.dram_tensor("wt", [128, 9 * 128], bf16, kind="ExternalInput").ap()
    out = nc.dram_tensor("out", [128, BLOC], bf16, kind="ExternalOutput").ap()

    with TileContext(nc) as tc:
        with (
            tc.tile_pool(name="const", bufs=1) as cpool,
            tc.tile_pool(name="work", bufs=2) as wpool,
            tc.tile_pool(name="obuf", bufs=6) as opool,
            tc.tile_pool(name="psum", bufs=8, space="PSUM") as ppool,
        ):
            # Dummy activation with no DMA dependencies: forces the
            # exp_and_others ACT table load to run during the preamble
            # instead of queueing behind the x DMAs.
            warm_act = cpool.tile([128, 1], f32, name="warm_act")
            nc.vector.memset(warm_act[:], 0.0)
            nc.scalar.activation(warm_act[:], warm_act[:], AF.Exp, scale=1.0)

            # PE HAM clock warm: memset-fed matmuls (no DMA deps) keep the
            # PE stream alive from t~0 so the clock gate is fully open
            # before the first real matmul.
            wm_s = cpool.tile([128, 128], bf16, name="wm_s")
            wm_m = cpool.tile([128, MMF], bf16, name="wm_m")
            nc.vector.memset(wm_s[:], 0.25)
            nc.vector.memset(wm_m[:], 0.25)
            warm_ps = ppool.tile([128, FDP], f32, name="warm_ps", tag="psum")
            for _ in range(12):
                nc.tensor.matmul(
                    warm_ps[:], wm_s[:], wm_m[:], start=True, stop=True
                )

            # weights first (300KB, ~1us) so matmuls are never
            # weight-blocked, then the x stream.
            w_sb = cpool.tile([128, 9, 128], bf16, name="w_sb")
            x_all = cpool.tile([128, BLOC], f32, name="x_all")
            xpieces = [512, 512, 1024] + [1024] * ((BLOC - 2048) // 1024)
            lo = 0
            for i, w in enumerate(xpieces):
                nc.sync.dma_start(x_all[:, lo : lo + w], xt[:, lo : lo + w])
                lo += w
                if i == 2:
                    # weights after the first chunk of x: x-dependent scalar
                    # work starts sooner; weights still beat the first
                    # weight-consuming matmul by a wide margin.
                    nc.sync.dma_start(
                        w_sb[:], wt.rearrange("p (g o) -> p g o", g=9)
                    )

            # Phase 1 (exp_and_others): s, q, t0 + vector chain + gauss MMs.
            psums = []
            last_t0 = None
            for c in range(NCH):
                s = wpool.tile([128, FDE], bf16, tag="s", bufs=3, name=f"s_{c}")
                q = wpool.tile([128, FDE], f32, tag="q", name=f"q_{c}")
                t0 = wpool.tile([128, FDE], bf16, tag="t0", bufs=3, name=f"t0_{c}")
                tg = [t0] + [
                    wpool.tile([128, FDE], bf16, tag=f"t{g}", name=f"t{g}_{c}")
                    for g in range(1, GRID)
                ]
                # chunk 0 runs at narrow widths so the chain starts earlier
                widths = [512, 512, 1024] if c == 0 else [FDE]
                off = 0
                for fd in widths:
                    hs = slice(off, off + fd)
                    xc = x_all[:, c * FDE + off : c * FDE + off + fd]
                    off += fd
                    nc.scalar.activation(s[:, hs], xc, AF.Exp, scale=2.0 / W)
                    nc.scalar.activation(
                        q[:, hs], xc, AF.Square, bias=1.0 / W, scale=1.0 / W
                    )
                    last_t0 = nc.scalar.activation(
                        t0[:, hs], q[:, hs], AF.Exp, scale=-1.0
                    )
                    for g in range(1, GRID):
                        nc.vector.tensor_mul(tg[g][:, hs], tg[g - 1][:, hs], s[:, hs])
                for half in range(FDE // FDP):
                    psum = ppool.tile([128, FDP], f32, tag="psum", name="psum")
                    psums.append(psum)
                    for g in range(GRID):
                        for n in range(FDP // MMF):
                            lo = half * FDP + n * MMF
                            nc.tensor.matmul(
                                psum[:, n * MMF : (n + 1) * MMF],
                                w_sb[:, g, :],
                                tg[g][:, lo : lo + MMF],
                                start=(g == 0),
                                stop=False,
                            )

            # Phase 2 (silu_and_others), gated on the last exp-phase act via a
            # bias tile so the two table phases cannot interleave on ScalarE.
            gate = cpool.tile([128, 1], f32)
            gate_op = nc.scalar.activation(
                gate[:], x_all[:, BLOC - 1 : BLOC], AF.Identity, scale=0.0
            )
            add_dep_helper(gate_op.ins, last_t0.ins, True, "table phase order")
            silu_all = cpool.tile([128, BLOC], bf16)
            FDS = 1024  # silu act width: finer so psum groups close sooner
            subs_per_silu = FDS // FDP

            def emit_copy(k, engine):
                ob = opool.tile([128, FDP], bf16, tag="ob", name=f"ob_{k}")
                if engine == "s":
                    nc.scalar.copy(ob[:], psums[k][:])
                else:
                    nc.vector.tensor_copy(ob[:], psums[k][:])
                nc.sync.dma_start(out[:, k * FDP : (k + 1) * FDP], ob[:])

            for j in range(BLOC // FDS):
                js = slice(j * FDS, (j + 1) * FDS)
                nc.scalar.activation(
                    silu_all[:, js], x_all[:, js], AF.Silu, bias=gate[:]
                )
                for half in range(subs_per_silu):
                    k = j * subs_per_silu + half
                    for n in range(FDP // MMF):
                        lo = k * FDP + n * MMF
                        nc.tensor.matmul(
                            psums[k][:, n * MMF : (n + 1) * MMF],
                            w_sb[:, 8, :],
                            silu_all[:, lo : lo + MMF],
                            start=False,
                            stop=True,
                        )
                # copies 0-3 free psum slots for the second-half gauss
                # matmuls, so they run early on ScalarE; the rest follow
                # the silu acts (ScalarE) or the chain (VectorE).
                if j == 1:
                    for k in range(0, 4):
                        emit_copy(k, "s")
            for k in range(4, 8):
                emit_copy(k, "s")
            for k in range(8, 12):
                emit_copy(k, "v")
            for k in range(12, NSUB):
                emit_copy(k, "s" if k % 2 == 0 else "v")


    nc.compile()
    return nc


def _prep_weights(coeffs, base_w):
    import ml_dtypes

    g = np.arange(GRID, dtype=np.float64)
    K = np.exp(7.0 * g - g * g)  # t_g = basis_g * e^(g^2-7g) -> fold inverse
    blocks = [
        (coeffs[:, :, gi].astype(np.float64) * K[gi]).T for gi in range(GRID)
    ]  # [in, out] each
    blocks.append(base_w.astype(np.float64).T)
    wtm = np.concatenate(blocks, axis=1)  # [128, 9*128]
    return np.ascontiguousarray(wtm.astype(ml_dtypes.bfloat16))


def kernel(x, coeffs, base_w, centers):
    from concourse.bass_utils import run_bass_kernel_spmd

    global _NC
    if _NC is None:
        _NC = _build()

    wtm = _prep_weights(coeffs, base_w)
    xT = np.ascontiguousarray(np.asarray(x, dtype=np.float32).T)  # [128, B]
    in_maps = [
        {
            "xt": np.ascontiguousarray(xT[:, c * BLOC : (c + 1) * BLOC]),
            "wt": wtm,
        }
        for c in range(NCORES)
    ]
    res = run_bass_kernel_spmd(_NC, in_maps, list(range(NCORES)))
    outT = np.concatenate(
        [res.results[c]["out"].astype(np.float32) for c in range(NCORES)], axis=1
    )
    return np.ascontiguousarray(outT.T)
